# revision 39
# baseline (speedup 1.0000x reference)
"""Trainium2 Bass kernel for nn_DeepSetLayer (GNN attention message passing).

Design (8 NeuronCores, graph-parallel by destination node):
  Host: append self-loops; bin-pack dst nodes into 8 cores x 50 blocks
  (<=128 dsts, <=13*128 edges per block); lay out three edge-ordered
  views of the node features (pure layout, no float math):
    XEM [edge, feat]  - src features, edge-major  (aggregation matmul lhsT)
    XTG [feat, edge]  - src features, feat-major  (per-edge q matmul lhsT)
    XDT [feat, edge]  - dst features, feat-major  (per-edge k matmul lhsT)
  Device (one SPMD program, no collectives):
    per 5-block group: 3 sequential DMA streams; per block:
      q_e = tanh(XTG.T@WqT + bq), k_e = XDT.T@WkT + bk   (PE matmuls with
        bias folded in via a ones-row matmul; tanh/copy on ACT)
      scores = sum(q*k)/sqrt(S), exp via ACT              (batched)
      per 128-edge tile: S_w[e,d] = (iota==dstloc)*exp_e  (one fused DVE op,
        bf16 everywhere for the DVE 4x perf mode)
        att_T += XEM_t.T @ S_w ; seg += S_w.T @ ones      (PE, PSUM accum)
      epilogue: xpre = (att_T.T@W2T)/seg + (x@W1T + b2), stored bf16;
        ssq accumulated via ACT Square (same act table as tanh/exp).
    phase 2 (after all blocks; single act-table switch to sqrt):
      rin = sqrt(1/ssq); out = relu(xpre * rin), streamed out per group.
  Host: inverse-permute per-core outputs into the full [N, F] result.
"""

import math
import sys

sys.path.insert(0, "/opt/trn_rl_repo")

import heapq

import ml_dtypes
import numpy as np

import concourse.bacc as bacc
import concourse.bass as bass
import concourse.mybir as mybir
import concourse.tile as tile
from concourse.bass_utils import run_bass_kernel_spmd

N = 50000
E = 600000
F = 128
S = 12
NCORES = 8
B = 50            # blocks per core
TB = 13           # 128-edge tiles per block
DSTS = B * 128    # 6400 padded dst slots per core
G = 5             # blocks per DMA group
NG = B // G
NPL = B * TB      # total edge tiles (planes) per core
GPL = G * TB      # planes per group
INV_SQRT_S = 1.0 / math.sqrt(float(S))
SWW = 132         # sw row stride per tile (128 dsts + 4 pad cols)
NE = TB * SWW + 4  # local_scatter table elems per block (even)

f32 = mybir.dt.float32
bf16 = mybir.dt.bfloat16
i16 = mybir.dt.int16
i32 = mybir.dt.int32
fp8 = mybir.dt.float8e4
bf16_np = ml_dtypes.bfloat16
fp8_np = ml_dtypes.float8_e4m3fn

_compiled = {}


def _pack_bins(deg):
    """Assign each dst to one of NCORES*B bins (<=128 dsts, <=TB*128 edges),
    balancing edge counts."""
    nbins = NCORES * B
    order = np.argsort(-deg, kind="stable")
    b_e = np.zeros(nbins, np.int64)
    b_n = np.zeros(nbins, np.int64)
    bins_dsts = [[] for _ in range(nbins)]
    heap = [(0, b) for b in range(nbins)]
    heapq.heapify(heap)
    for dst in order:
        dst = int(dst)
        d = int(deg[dst])
        stash = []
        while True:
            ec, b = heapq.heappop(heap)
            if ec != b_e[b]:
                continue
            if b_n[b] < 128 and b_e[b] + d <= TB * 128:
                break
            stash.append((ec, b))
        bins_dsts[b].append(dst)
        b_e[b] += d
        b_n[b] += 1
        if b_n[b] < 128:
            heapq.heappush(heap, (int(b_e[b]), b))
        for it in stash:
            heapq.heappush(heap, it)
    return bins_dsts


def _host_prep(node_data, src, dst):
    x = np.ascontiguousarray(np.asarray(node_data, np.float32))
    loops = np.arange(N, dtype=np.int64)
    s_all = np.concatenate([np.asarray(src, np.int64), loops])
    d_all = np.concatenate([np.asarray(dst, np.int64), loops])

    deg = np.bincount(d_all, minlength=N)
    bins_dsts = _pack_bins(deg)

    perm = np.full(NCORES * DSTS, -1, dtype=np.int64)
    for b, dlist in enumerate(bins_dsts):
        core, blk = divmod(b, B)
        base = core * DSTS + blk * 128
        perm[base : base + len(dlist)] = dlist

    # CSR of edges by dst
    eorder = np.argsort(d_all, kind="stable")
    indptr = np.zeros(N + 1, dtype=np.int64)
    np.cumsum(deg, out=indptr[1:])
    s_sorted = s_all[eorder]

    node_bf = np.ascontiguousarray(x.astype(bf16_np))

    per_core = []
    for core in range(NCORES):
        srcmat = np.zeros((128, NPL), np.int64)
        dstmat = np.zeros((128, NPL), np.int64)
        dstloc = np.full((128, NPL), -1, dtype=np.int64)

        for blk in range(B):
            dlist = bins_dsts[core * B + blk]
            ss, dd_, dl = [], [], []
            for j, d0 in enumerate(dlist):
                es = s_sorted[indptr[d0] : indptr[d0 + 1]]
                ss.append(es)
                dd_.append(np.full(len(es), d0, np.int64))
                dl.append(np.full(len(es), j, np.int64))
            ss = np.concatenate(ss) if ss else np.zeros(0, np.int64)
            dd_ = np.concatenate(dd_) if dd_ else np.zeros(0, np.int64)
            dl = np.concatenate(dl) if dl else np.zeros(0, np.int64)
            ne = len(ss)
            assert ne <= TB * 128, f"block overflow {ne}"
            sp = np.zeros(TB * 128, np.int64)
            sp[:ne] = ss
            dp = np.zeros(TB * 128, np.int64)
            dp[:ne] = dd_
            lp = np.full(TB * 128, -1, np.int64)
            lp[:ne] = dl
            # edge slot j -> (plane j//128, partition j%128)
            pl0 = blk * TB
            srcmat[:, pl0 : pl0 + TB] = sp.reshape(TB, 128).T
            dstmat[:, pl0 : pl0 + TB] = dp.reshape(TB, 128).T
            dstloc[:, pl0 : pl0 + TB] = lp.reshape(TB, 128).T

        # local_scatter indices: [128, B, 14] int16; col t<TB holds
        # t*SWW + dstloc (pad edges -> t*SWW-1: ignored at t=0, dump col
        # otherwise); col 13 is -1 (ignored).
        idx16 = np.full((128, B, 14), -1, np.int16)
        for blk in range(B):
            dl_blk = dstloc[:, blk * TB : (blk + 1) * TB]  # [128, TB]
            idx16[:, blk, :TB] = (
                np.arange(TB, dtype=np.int64)[None, :] * SWW + dl_blk
            ).astype(np.int16)
        idx16 = np.ascontiguousarray(idx16.reshape(128, B * 14))

        # edge-ordered feature streams
        g_src = node_bf[srcmat]                 # [128 e, NPL, F]
        g_dst = node_bf[dstmat]                 # [128 e, NPL, F]
        xem = np.ascontiguousarray(g_src.reshape(128, NPL * F))
        xtg = np.ascontiguousarray(
            g_src.transpose(2, 1, 0).reshape(128, NPL * 128)
        ).astype(fp8_np)  # [feat, (plane, edge)]
        xdt = np.ascontiguousarray(
            g_dst.transpose(2, 1, 0).reshape(128, NPL * 128)
        ).astype(fp8_np)

        nshT = np.zeros((F, DSTS), bf16_np)
        sl = perm[core * DSTS : (core + 1) * DSTS]
        valid = sl >= 0
        nshT[:, valid] = node_bf[sl[valid]].T

        per_core.append(
            dict(xem=xem, xtg=xtg, xdt=xdt, nshT=nshT, idx16=idx16)
        )

    return per_core, perm


def _build_nc():
    nc = bacc.Bacc(
        "TRN2",
        target_bir_lowering=False,
        debug=False,
        enable_asserts=False,
        num_devices=NCORES,
    )
    AF = mybir.ActivationFunctionType
    OP = mybir.AluOpType

    xem_d = nc.dram_tensor("xem", [128, NPL * F], bf16, kind="ExternalInput")
    xtg_d = nc.dram_tensor("xtg", [128, NPL * F], fp8, kind="ExternalInput")
    xdt_d = nc.dram_tensor("xdt", [128, NPL * F], fp8, kind="ExternalInput")
    nshT_d = nc.dram_tensor("nshT", [F, DSTS], bf16, kind="ExternalInput")
    idx16_d = nc.dram_tensor("idx16", [128, B * 14], i16, kind="ExternalInput")
    wqT_d = nc.dram_tensor("wqT", [F, S], fp8, kind="ExternalInput")
    wkT_d = nc.dram_tensor("wkT", [F, S], fp8, kind="ExternalInput")
    w1T_d = nc.dram_tensor("w1T", [F, F], bf16, kind="ExternalInput")
    w2T_d = nc.dram_tensor("w2T", [F, F], bf16, kind="ExternalInput")
    bqr_d = nc.dram_tensor("bqr", [1, TB * S], bf16, kind="ExternalInput")
    bkr_d = nc.dram_tensor("bkr", [1, TB * S], bf16, kind="ExternalInput")
    b2r_d = nc.dram_tensor("b2r", [1, F], bf16, kind="ExternalInput")
    out_d = nc.dram_tensor("out", [DSTS, F], f32, kind="ExternalOutput")

    with tile.TileContext(nc) as tc:
        with tc.tile_pool(name="const", bufs=1) as const:
            wqT = const.tile([F, S], fp8)
            nc.sync.dma_start(wqT[:], wqT_d[:])
            wkT = const.tile([F, S], fp8)
            nc.sync.dma_start(wkT[:], wkT_d[:])
            idx16 = const.tile([128, B * 14], i16)
            nc.sync.dma_start(idx16[:], idx16_d[:])
            # nshT rides the gpsimd SWDGE ring so the first XEM group isn't
            # queued behind 1.6MB of constants on the sync HWDGE ring
            nshT = const.tile([F, DSTS], bf16)
            nc.gpsimd.dma_start(nshT[:], nshT_d[:])
            w1T = const.tile([F, F], bf16)
            nc.sync.dma_start(w1T[:], w1T_d[:])
            w2T = const.tile([F, F], bf16)
            nc.sync.dma_start(w2T[:], w2T_d[:])
            bqr = const.tile([1, TB * S], bf16)
            nc.sync.dma_start(bqr[:], bqr_d[:])
            bkr = const.tile([1, TB * S], bf16)
            nc.sync.dma_start(bkr[:], bkr_d[:])
            b2r = const.tile([1, F], bf16)
            nc.sync.dma_start(b2r[:], b2r_d[:])
            ones_col = const.tile([128, 1], bf16)
            nc.vector.memset(ones_col[:], 1.0)
            ones_row = const.tile([1, 128], bf16)
            nc.vector.memset(ones_row[:], 1.0)
            # bf16 xpre for all blocks (phase-2 normalization), plus ssq
            xpre_all = const.tile([128, B, F], bf16)
            ssq_all = const.tile([128, B], f32)
            # exp values for local_scatter, all blocks; zero the pad column
            # once (col 13 is read by every scatter, idx -1 ignores it)
            expw_all = const.tile([128, B, 14], bf16)
            nc.gpsimd.memset(expw_all[:], 0.0)

            with (
                tc.tile_pool(name="xe", bufs=3) as xep,
                tc.tile_pool(name="xt", bufs=3) as xtp,
                tc.tile_pool(name="xd", bufs=3) as xdp,
                tc.tile_pool(name="wk3", bufs=3) as wk3,
                tc.tile_pool(name="swp", bufs=4) as swp,
                tc.tile_pool(name="outp", bufs=2) as outp,
                tc.tile_pool(name="p2", bufs=2) as p2,
                tc.tile_pool(name="ps_q", bufs=2, space="PSUM") as ps_q,
                tc.tile_pool(name="ps_k", bufs=1, space="PSUM") as ps_k,
                tc.tile_pool(name="ps_att", bufs=2, space="PSUM") as ps_att,
                tc.tile_pool(name="ps_seg", bufs=1, space="PSUM") as ps_seg,
                tc.tile_pool(name="ps_ab", bufs=1, space="PSUM") as ps_ab,
            ):
                # deferred PE epilogue of the previous block (software
                # pipelining: keeps the in-order PE queue from stalling on
                # the ACT att-copy)
                def pe_epilogue(b, attT_sb, rec):
                    pa = ps_ab.tile([128, F], f32, tag="pa")
                    nc.tensor.matmul(
                        pa[:], attT_sb[:], w2T[:], start=True, stop=True
                    )
                    pb = ps_ab.tile([128, F], f32, tag="pb")
                    nc.tensor.matmul(
                        pb[:],
                        nshT[:, b * 128 : (b + 1) * 128],
                        w1T[:],
                        start=True,
                        stop=False,
                    )
                    nc.tensor.matmul(
                        pb[:], ones_row[:], b2r[:], start=False, stop=True
                    )
                    # attn = pa/seg on ACT (scale = per-partition rec AP);
                    # then xpre = attn + pb with a single PSUM operand.
                    attn_sb = wk3.tile([128, F], bf16, tag="attn")
                    nc.scalar.activation(
                        attn_sb[:], pa[:], AF.Copy, scale=rec[:]
                    )
                    nc.vector.tensor_tensor(
                        xpre_all[:, b, :], attn_sb[:], pb[:], OP.add
                    )
                    # ssq via DVE STT square+accum (bf16 SBUF, 2x mode)
                    sqd = wk3.tile([128, F], bf16, tag="sqd")
                    nc.vector.scalar_tensor_tensor(
                        sqd[:], xpre_all[:, b, :], 1.0, xpre_all[:, b, :],
                        OP.mult, OP.mult,
                        accum_out=ssq_all[:, b : b + 1],
                    )

                pend = None
                for g in range(NG):
                    csl = slice(g * GPL * F, (g + 1) * GPL * F)
                    h = GPL * F // 2
                    # q path consumes XTG first; issue it first (split in two
                    # for faster first-tile availability) and spread the three
                    # streams over both HWDGE rings (SP + ACT).
                    XTG = xtp.tile([128, GPL * F], fp8, tag="XTG")
                    nc.scalar.dma_start(
                        XTG[:, 0:h], xtg_d[:, csl.start : csl.start + h]
                    )
                    nc.scalar.dma_start(
                        XTG[:, h:], xtg_d[:, csl.start + h : csl.stop]
                    )
                    XDT = xdp.tile([128, GPL * F], fp8, tag="XDT")
                    nc.scalar.dma_start(XDT[:], xdt_d[:, csl])
                    XEM = xep.tile([128, GPL * F], bf16, tag="XEM")
                    nc.sync.dma_start(XEM[:], xem_d[:, csl])

                    for bb in range(G):
                        b = g * G + bb
                        psq = ps_q.tile([128, TB, S], f32, tag="psq")
                        psk = ps_k.tile([128, TB, S], f32, tag="psk")
                        for t in range(TB):
                            fsl = slice((bb * TB + t) * F, (bb * TB + t + 1) * F)
                            nc.tensor.matmul(
                                psq[:, t, :], XTG[:, fsl], wqT[:],
                                start=(t == 0), stop=False,
                            )
                            nc.tensor.matmul(
                                psk[:, t, :], XDT[:, fsl], wkT[:],
                                start=(t == 0), stop=False,
                            )
                        # bias via ones-row matmul over the whole block
                        nc.tensor.matmul(
                            psq[:].rearrange("p t s -> p (t s)"),
                            ones_row[:],
                            bqr[:],
                            start=False, stop=True,
                        )
                        nc.tensor.matmul(
                            psk[:].rearrange("p t s -> p (t s)"),
                            ones_row[:],
                            bkr[:],
                            start=False, stop=True,
                        )
                        q_sb = wk3.tile([128, TB, S], bf16, tag="qsb")
                        nc.scalar.activation(q_sb[:], psq[:], AF.Tanh)
                        k_sb = wk3.tile([128, TB, S], bf16, tag="ksb")
                        nc.vector.tensor_copy(k_sb[:], psk[:])
                        prod = wk3.tile([128, TB, S], bf16, tag="prod")
                        nc.vector.tensor_tensor(prod[:], q_sb[:], k_sb[:], OP.mult)
                        scores = wk3.tile([128, TB], f32, tag="scores")
                        nc.vector.tensor_reduce(
                            scores[:], prod[:], mybir.AxisListType.X, OP.add
                        )
                        nc.scalar.activation(
                            expw_all[:, b, 0:TB], scores[:], AF.Exp,
                            scale=INV_SQRT_S,
                        )

                        # S_w for the whole block via gpsimd local_scatter:
                        # zeroes [128, NE], writes exp at t*SWW + dstloc.
                        sw = swp.tile([128, NE], bf16, tag="sw")
                        nc.gpsimd.local_scatter(
                            sw[:],
                            expw_all[:, b, :],
                            idx16[:, b * 14 : (b + 1) * 14],
                            channels=128,
                            num_elems=NE,
                            num_idxs=14,
                        )
                        att = ps_att.tile([F, 128], f32, tag="att")
                        seg = ps_seg.tile([128, 1], f32, tag="seg")
                        for t in range(TB):
                            fsl = slice((bb * TB + t) * F, (bb * TB + t + 1) * F)
                            swt = sw[:, t * SWW : t * SWW + 128]
                            nc.tensor.matmul(
                                att[:], XEM[:, fsl], swt,
                                start=(t == 0), stop=(t == TB - 1),
                            )
                            nc.tensor.matmul(
                                seg[:], swt, ones_col[:],
                                start=(t == 0), stop=(t == TB - 1),
                            )

                        attT_sb = wk3.tile([F, 128], bf16, tag="attTsb")
                        nc.scalar.activation(attT_sb[:], att[:], AF.Copy)
                        rec = wk3.tile([128, 1], f32, tag="rec")
                        nc.vector.reciprocal(rec[:], seg[:])
                        if pend is not None:
                            pe_epilogue(*pend)
                        pend = (b, attT_sb, rec)

                pe_epilogue(*pend)

                # ---- phase 2 (end): DVE rsqrt over all blocks (bit-trick
                # seed + 2 Newton steps, no act-table switch), then
                # relu(xpre*rin) split across ACT and DVE in parallel.
                yt = p2.tile([128, B], i32, tag="yt")
                nc.vector.tensor_scalar(
                    yt[:], ssq_all[:].bitcast(i32), 1, None,
                    OP.logical_shift_right,
                )
                y0 = p2.tile([128, B], i32, tag="y0")
                nc.vector.tensor_scalar(
                    y0[:], yt[:], -1, 0x5F3759DF, OP.mult, OP.add
                )
                h = p2.tile([128, B], f32, tag="h")
                nc.vector.tensor_scalar(h[:], ssq_all[:], -0.5, None, OP.mult)
                y = y0[:].bitcast(f32)
                for _ in range(2):
                    y2 = p2.tile([128, B], f32, tag="y2")
                    nc.vector.tensor_tensor(y2[:], y, y, OP.mult)
                    t2 = p2.tile([128, B], f32, tag="t2")
                    nc.vector.tensor_tensor(t2[:], y2[:], h[:], OP.mult)
                    t3 = p2.tile([128, B], f32, tag="t3")
                    nc.vector.tensor_scalar(t3[:], t2[:], 1.5, None, OP.add)
                    yn = p2.tile([128, B], f32, tag="yn")
                    nc.vector.tensor_tensor(yn[:], y, t3[:], OP.mult)
                    y = yn[:]
                for g in range(NG):
                    ot = outp.tile([128, G, F], f32, tag="ot")
                    for bb in range(G):
                        b = g * G + bb
                        if bb % 2 == 0:
                            nc.scalar.activation(
                                ot[:, bb, :], xpre_all[:, b, :], AF.Relu,
                                scale=yn[:, b : b + 1],
                            )
                        else:
                            nc.vector.tensor_scalar(
                                ot[:, bb, :], xpre_all[:, b, :],
                                yn[:, b : b + 1], 0.0, OP.mult, OP.max,
                            )
                    eng = nc.sync if g % 2 == 0 else nc.scalar
                    eng.dma_start(
                        out_d[g * G * 128 : (g + 1) * G * 128, :].rearrange(
                            "(j p) f -> p j f", p=128
                        ),
                        ot[:],
                    )

    nc.compile()
    return nc


def get_nc():
    if "nc" not in _compiled:
        _compiled["nc"] = _build_nc()
    return _compiled["nc"]


def _make_in_maps(node_data, src, dst, Wq, bq, Wk, bk, W1, W2, b2):
    per_core, perm = _host_prep(node_data, src, dst)
    consts = dict(
        wqT=np.ascontiguousarray(np.asarray(Wq, np.float32).T).astype(fp8_np),
        wkT=np.ascontiguousarray(np.asarray(Wk, np.float32).T).astype(fp8_np),
        w1T=np.ascontiguousarray(np.asarray(W1, np.float32).T).astype(bf16_np),
        w2T=np.ascontiguousarray(np.asarray(W2, np.float32).T).astype(bf16_np),
        bqr=np.tile(np.asarray(bq, np.float32), TB)[None, :].astype(bf16_np),
        bkr=np.tile(np.asarray(bk, np.float32), TB)[None, :].astype(bf16_np),
        b2r=np.asarray(b2, np.float32)[None, :].astype(bf16_np),
    )
    in_maps = []
    for core in range(NCORES):
        m = dict(consts)
        m.update(per_core[core])
        in_maps.append(m)
    return in_maps, perm


def run(node_data, src, dst, Wq, bq, Wk, bk, W1, W2, b2, trace=False):
    in_maps, perm = _make_in_maps(
        node_data, src, dst, Wq, bq, Wk, bk, W1, W2, b2
    )
    nc = get_nc()
    res = run_bass_kernel_spmd(nc, in_maps, list(range(NCORES)), trace=trace)
    out = np.zeros((N, F), dtype=np.float32)
    for core in range(NCORES):
        o = np.asarray(res.results[core]["out"], np.float32)
        sl = perm[core * DSTS : (core + 1) * DSTS]
        valid = sl >= 0
        out[sl[valid]] = o[valid]
    return out, res


def kernel(node_data, src, dst, Wq, bq, Wk, bk, W1, W2, b2):
    out, _ = run(node_data, src, dst, Wq, bq, Wk, bk, W1, W2, b2, trace=False)
    return out


if __name__ == "__main__":
    nc = get_nc()
    print("compiled OK")
